# revision 7
# baseline (speedup 1.0000x reference)
"""Multi-head attention + residual + LayerNorm on 8 Trainium2 NeuronCores.

Problem: nn_MultiHeadAttention (B=2, S=2048, D=768, H=12, causal mask),
reference returns (out [2,2048,768], attn [2,12,2048,2048]) in fp32.

Sharding: core = (batch b, 4 q-row tiles of 128). The 16 q-tiles of each batch
are assigned to 4 cores as {j, 7-j, 8+j, 15-j} so causal work/writes balance.
Each core computes all 12 heads for its 512 q rows: Q/K/V projections
(K/V for the full sequence, replicated within a batch group), scores,
softmax, the attn probability tiles (written only for the causal region;
the exact-zero upper triangle is assembled host-side), attn@V, output
projection, residual add and LayerNorm for its rows.

Compute dtype: bf16 operands with fp32 PSUM accumulation; softmax in fp32.

One NEFF runs on all 8 cores (SPMD). Per-core tile sets {t0, 7-t0, 8+t0,
15-t0} (t0 in 0..3) have position-wise worst cases TMAX=[3,7,11,15]; the
program computes worst-case causal widths LMAX per position and host-supplied
additive masks (0 keep / -1e30 drop) encode each core's true tril boundary,
so surplus columns exp() to exact zeros (correct attn output values, zero
contributions to attn@V). Only the last 4 key-blocks of each position can
differ per core, so the masks are small: [128,4,512] forward, [128,4,4,128]
for the transposed path. The reference causal mask is deterministic, so the
`mask` input itself is not re-read on device.
"""

from contextlib import ExitStack

import ml_dtypes
import numpy as np

import concourse.tile as tile
from concourse import bacc, mybir
from concourse.bass_utils import run_bass_kernel_spmd
from concourse.masks import make_identity

B, S, D, H, DK = 2, 2048, 768, 12, 64
NT = S // 128  # 16 q tiles per batch
EPS = 1e-5
N_CORES = 8
NEG = -1.0e30

F32 = mybir.dt.float32
BF16 = mybir.dt.bfloat16
BF16_NP = ml_dtypes.bfloat16
AX = mybir.AxisListType
ALU = mybir.AluOpType
AF = mybir.ActivationFunctionType

TMAX = [3, 7, 11, 15]  # worst-case tile index per position
LMAX = [(t + 1) * 128 for t in TMAX]  # 512, 1024, 1536, 2048


def tile_set(j):
    return sorted([j, 7 - j, 8 + j, 15 - j])


def build_nc_real():
    nc = bacc.Bacc("TRN2", target_bir_lowering=False, debug=False,
                   num_devices=N_CORES)

    qT_d = nc.dram_tensor("qT", [D, 512], BF16, kind="ExternalInput")
    kT_d = nc.dram_tensor("kT", [D, S], BF16, kind="ExternalInput")
    vT_d = nc.dram_tensor("vT", [D, S], BF16, kind="ExternalInput")
    qres_d = nc.dram_tensor("q_res", [512, D], F32, kind="ExternalInput")
    WqT_d = nc.dram_tensor("WqT", [D, D], BF16, kind="ExternalInput")
    WkT_d = nc.dram_tensor("WkT", [D, D], BF16, kind="ExternalInput")
    WvT_d = nc.dram_tensor("WvT", [D, D], BF16, kind="ExternalInput")
    WoT_d = nc.dram_tensor("WoT", [D, D], BF16, kind="ExternalInput")
    bqp_d = nc.dram_tensor("bq_p", [128, 6], F32, kind="ExternalInput")
    bkp_d = nc.dram_tensor("bk_p", [128, 6], F32, kind="ExternalInput")
    bvb_d = nc.dram_tensor("bv_b", [128, D], F32, kind="ExternalInput")
    gam_d = nc.dram_tensor("gamma_b", [128, D], F32, kind="ExternalInput")
    bet_d = nc.dram_tensor("beta_b", [128, D], F32, kind="ExternalInput")
    # forward additive mask: per position, last 512 score columns
    am4_d = nc.dram_tensor("am4", [128, 4, 512], F32, kind="ExternalInput")
    # transposed-path 0/1 multiplicative mask (bf16), per (position, joff),
    # duplicated for the two heads of a pair: [128, pos, joff, 2, 128]
    amT4_d = nc.dram_tensor("amT4", [128, 4, 4, 2, 128], BF16, kind="ExternalInput")

    attn_d = nc.dram_tensor("attn_o", [H, 512, S], F32, kind="ExternalOutput")
    out_d = nc.dram_tensor("out_o", [512, D], F32, kind="ExternalOutput")

    with ExitStack() as top:
        tc = top.enter_context(tile.TileContext(nc))

        consts = top.enter_context(tc.tile_pool(name="consts", bufs=1))
        persist = top.enter_context(tc.tile_pool(name="persist", bufs=1))
        sp = top.enter_context(tc.tile_pool(name="sp", bufs=2, space="PSUM"))
        etp = top.enter_context(tc.tile_pool(name="etp", bufs=2, space="PSUM"))
        stats = top.enter_context(tc.tile_pool(name="stats", bufs=16))

        # ---- consts ----
        WoT_sb = consts.tile([128, 6, D], BF16, tag="wot")
        nc.sync.dma_start(out=WoT_sb, in_=WoT_d.ap().rearrange("(c p) d -> p c d", p=128))
        bq_sb = consts.tile([128, 6], F32, tag="bq")
        nc.sync.dma_start(out=bq_sb, in_=bqp_d.ap())
        bk_sb = consts.tile([128, 6], F32, tag="bk")
        nc.sync.dma_start(out=bk_sb, in_=bkp_d.ap())
        bvb_sb = consts.tile([128, D], F32, tag="bvb")
        nc.sync.dma_start(out=bvb_sb, in_=bvb_d.ap())
        gam_sb = consts.tile([128, D], F32, tag="gam")
        nc.sync.dma_start(out=gam_sb, in_=gam_d.ap())
        bet_sb = consts.tile([128, D], F32, tag="bet")
        nc.sync.dma_start(out=bet_sb, in_=bet_d.ap())
        am4_sb = consts.tile([128, 4, 512], F32, tag="am4")
        nc.sync.dma_start(out=am4_sb, in_=am4_d.ap())
        amT4_sb = consts.tile([128, 4, 4, 2, 128], BF16, tag="amT4")
        nc.sync.dma_start(out=amT4_sb, in_=amT4_d.ap())
        ident_sb = consts.tile([128, 128], BF16, tag="ident")
        make_identity(nc, ident_sb)
        eps_sb = consts.tile([128, 1], F32, tag="eps")
        nc.vector.memset(eps_sb, EPS)
        r_all = consts.tile([128, H, 4], F32, tag="rall")

        # ---- persistent activations ----
        QT_sb = [persist.tile([128, 512], BF16, tag=f"qt{p}", name=f"QT{p}")
                 for p in range(6)]
        KT_sb = [persist.tile([128, S], BF16, tag=f"kt{p}", name=f"KT{p}")
                 for p in range(6)]
        V_sb = [persist.tile([128, D], BF16, tag=f"v{i}", name=f"V{i}")
                for i in range(NT)]
        O_sb = [persist.tile([128, D], BF16, tag=f"o{i}", name=f"O{i}")
                for i in range(4)]

        # ================= phase A: projections =================
        with tc.tile_pool(name="pha", bufs=1) as pha, \
             tc.tile_pool(name="phs", bufs=3) as phs:
            Wq_sb = pha.tile([128, 6, D], BF16, tag="wq")
            nc.sync.dma_start(out=Wq_sb, in_=WqT_d.ap().rearrange("(c p) d -> p c d", p=128))
            Wk_sb = pha.tile([128, 6, D], BF16, tag="wk")
            nc.sync.dma_start(out=Wk_sb, in_=WkT_d.ap().rearrange("(c p) d -> p c d", p=128))
            Wv_sb = pha.tile([128, 6, D], BF16, tag="wv")
            nc.sync.dma_start(out=Wv_sb, in_=WvT_d.ap().rearrange("(c p) d -> p c d", p=128))
            qTi = pha.tile([128, 6, 512], BF16, tag="qTi")
            nc.sync.dma_start(out=qTi, in_=qT_d.ap().rearrange("(c p) n -> p c n", p=128))

            for p in range(6):
                ps = sp.tile([128, 1024], F32, tag="sp", name=f"qps{p}")
                for c in range(6):
                    nc.tensor.matmul(ps[:, 0:512],
                                     lhsT=Wq_sb[:, c, p * 128:(p + 1) * 128],
                                     rhs=qTi[:, c, :],
                                     start=(c == 0), stop=(c == 5))
                nc.scalar.activation(out=QT_sb[p], in_=ps[:, 0:512],
                                     func=AF.Identity, bias=bq_sb[:, p:p + 1])
            for ch in range(4):
                kTi = phs.tile([128, 6, 512], BF16, tag="kTi", name=f"kTi{ch}")
                nc.sync.dma_start(
                    out=kTi,
                    in_=kT_d.ap().rearrange("(c p) n -> p c n", p=128)[:, :, ch * 512:(ch + 1) * 512])
                for p in range(6):
                    ps = sp.tile([128, 1024], F32, tag="sp", name=f"kps{ch}_{p}")
                    for c in range(6):
                        nc.tensor.matmul(ps[:, 0:512],
                                         lhsT=Wk_sb[:, c, p * 128:(p + 1) * 128],
                                         rhs=kTi[:, c, :],
                                         start=(c == 0), stop=(c == 5))
                    nc.scalar.activation(out=KT_sb[p][:, ch * 512:(ch + 1) * 512],
                                         in_=ps[:, 0:512],
                                         func=AF.Identity, bias=bk_sb[:, p:p + 1])
            for ii in range(NT // 2):
                vTi = phs.tile([128, 6, 256], BF16, tag="vTi", name=f"vTi{ii}")
                nc.sync.dma_start(
                    out=vTi,
                    in_=vT_d.ap().rearrange("(c p) n -> p c n", p=128)[:, :, ii * 256:(ii + 1) * 256])
                for s2 in range(2):
                    i = 2 * ii + s2
                    ps = sp.tile([128, 1024], F32, tag="sp", name=f"vps{i}")
                    for c in range(6):
                        nc.tensor.matmul(ps[:, 0:512],
                                         lhsT=vTi[:, c, s2 * 128:(s2 + 1) * 128],
                                         rhs=Wv_sb[:, c, 0:512],
                                         start=(c == 0), stop=(c == 5))
                        nc.tensor.matmul(ps[:, 512:768],
                                         lhsT=vTi[:, c, s2 * 128:(s2 + 1) * 128],
                                         rhs=Wv_sb[:, c, 512:768],
                                         start=(c == 0), stop=(c == 5))
                    nc.vector.tensor_add(V_sb[i], ps[:, 0:768], bvb_sb)

        # ================= phase B: per head-pair =================
        etpool = top.enter_context(tc.tile_pool(name="etpool", bufs=1))
        efwd = top.enter_context(tc.tile_pool(name="efwd", bufs=3))
        cpool = top.enter_context(tc.tile_pool(name="cpool", bufs=2))

        for p in range(6):
            heads = (2 * p, 2 * p + 1)
            # ---- forward scores + softmax + attn DMA ----
            for hi, h in enumerate(heads):
                for pos in range(4):
                    Lm = LMAX[pos]
                    efw = efwd.tile([128, S], F32, tag="efwd", name=f"ef{h}_{pos}")
                    parts = stats.tile([128, 2], F32, tag="st", name=f"pt{h}_{pos}")
                    nchunk = (Lm + 1023) // 1024
                    for cch in range(nchunk):
                        cl = min(1024, Lm - cch * 1024)
                        ps = sp.tile([128, 1024], F32, tag="sp",
                                     name=f"sps{h}_{pos}_{cch}")
                        for cc in range((cl + 511) // 512):
                            w = min(512, cl - cc * 512)
                            col = cch * 1024 + cc * 512
                            nc.tensor.matmul(
                                ps[:, cc * 512:cc * 512 + w],
                                lhsT=QT_sb[p][hi * 64:(hi + 1) * 64,
                                              pos * 128:(pos + 1) * 128],
                                rhs=KT_sb[p][hi * 64:(hi + 1) * 64, col:col + w],
                                start=True, stop=True)
                        if cch == nchunk - 1:
                            # causal mask on the last 512 columns
                            nc.vector.tensor_add(ps[:, cl - 512:cl],
                                                 ps[:, cl - 512:cl],
                                                 am4_sb[:, pos, :])
                        nc.scalar.activation(out=efw[:, cch * 1024:cch * 1024 + cl],
                                             in_=ps[:, 0:cl], func=AF.Exp,
                                             scale=0.125,
                                             accum_out=parts[:, cch:cch + 1])
                    if nchunk == 2:
                        nc.vector.tensor_add(parts[:, 0:1], parts[:, 0:1],
                                             parts[:, 1:2])
                    rr = r_all[:, h, pos:pos + 1]
                    nc.vector.reciprocal(out=rr, in_=parts[:, 0:1])
                    nc.vector.tensor_scalar_mul(out=efw[:, 0:Lm],
                                                in0=efw[:, 0:Lm], scalar1=rr)
                    nc.gpsimd.dma_start(out=attn_d[h, pos * 128:(pos + 1) * 128, 0:Lm],
                                        in_=efw[:, 0:Lm])

            # ---- transposed exp tiles (ET) ----
            ets = []
            for j in range(NT):
                i0 = j // 4  # first position with TMAX >= j
                nv = (4 - i0) * 128
                ps = etp.tile([128, 2, 512], F32, tag="etp", name=f"etp{p}_{j}")
                for hi in range(2):
                    nc.tensor.matmul(ps[:, hi, 0:nv],
                                     lhsT=KT_sb[p][hi * 64:(hi + 1) * 64,
                                                   j * 128:(j + 1) * 128],
                                     rhs=QT_sb[p][hi * 64:(hi + 1) * 64,
                                                  i0 * 128:512],
                                     start=True, stop=True)
                et = etpool.tile([128, 2, nv], BF16, tag=f"etq{i0}",
                                 name=f"et{p}_{j}", bufs=5)
                nc.scalar.activation(out=et, in_=ps[:, :, 0:nv],
                                     func=AF.Exp, scale=0.125)
                # per-core causal 0/1 mask: only block i = j//4 can differ
                nc.vector.tensor_mul(et[:, :, 0:128], et[:, :, 0:128],
                                     amT4_sb[:, i0, j - 4 * i0, :, :])
                ets.append((et, i0))

            # ---- attn @ V (forward orientation, unnormalized) ----
            for hi, h in enumerate(heads):
                ops = sp.tile([128, 4, 64], F32, tag="sp", name=f"ops{h}")
                for pos in range(4):
                    tmax = TMAX[pos]
                    for j in range(tmax + 1):
                        et, i0 = ets[j]
                        blk = (pos - i0) * 128
                        nc.tensor.matmul(ops[:, pos, :],
                                         lhsT=et[:, hi, blk:blk + 128],
                                         rhs=V_sb[j][:, h * 64:(h + 1) * 64],
                                         start=(j == 0), stop=(j == tmax))
                for pos in range(4):
                    nc.vector.tensor_scalar_mul(
                        out=O_sb[pos][:, h * 64:(h + 1) * 64],
                        in0=ops[:, pos, :],
                        scalar1=r_all[:, h, pos:pos + 1])

        # ================= phase C: out-proj + residual + LN =================
        for pos in range(4):
            ot_sb = []
            for pch in range(6):
                tp = sp.tile([128, 128], BF16, tag="sp", name=f"tp{pos}_{pch}")
                nc.tensor.transpose(out=tp,
                                    in_=O_sb[pos][:, pch * 128:(pch + 1) * 128],
                                    identity=ident_sb)
                ot = cpool.tile([128, 128], BF16, tag=f"ot{pch}",
                                name=f"ot{pos}_{pch}")
                nc.vector.tensor_copy(out=ot, in_=tp)
                ot_sb.append(ot)
            ps = sp.tile([128, 1024], F32, tag="sp", name=f"prj{pos}")
            for pch in range(6):
                nc.tensor.matmul(ps[:, 0:512], lhsT=ot_sb[pch],
                                 rhs=WoT_sb[:, pch, 0:512],
                                 start=(pch == 0), stop=(pch == 5))
                nc.tensor.matmul(ps[:, 512:768], lhsT=ot_sb[pch],
                                 rhs=WoT_sb[:, pch, 512:768],
                                 start=(pch == 0), stop=(pch == 5))
            qr = cpool.tile([128, D], F32, tag="qr", name=f"qr{pos}")
            nc.sync.dma_start(out=qr, in_=qres_d[pos * 128:(pos + 1) * 128, :])
            x = cpool.tile([128, D], F32, tag="x", name=f"x{pos}")
            nc.vector.tensor_add(x, ps[:, 0:768], qr)
            nsum = stats.tile([128, 1], F32, tag="nsum", name=f"ns{pos}")
            nc.vector.tensor_reduce(out=nsum, in_=x, axis=AX.X, op=ALU.add,
                                    negate=True)
            nmean = stats.tile([128, 1], F32, tag="nmean", name=f"nm{pos}")
            nc.vector.tensor_scalar_mul(out=nmean, in0=nsum, scalar1=1.0 / D)
            xc = cpool.tile([128, D], F32, tag="xc", name=f"xc{pos}")
            nc.scalar.activation(out=xc, in_=x, func=AF.Identity, bias=nmean)
            xsq = cpool.tile([128, D], F32, tag="xg", name=f"xq{pos}")
            vs = stats.tile([128, 1], F32, tag="vs", name=f"vs{pos}")
            nc.scalar.activation(out=xsq, in_=xc, func=AF.Square,
                                 accum_out=vs)
            std = stats.tile([128, 1], F32, tag="std", name=f"sd{pos}")
            nc.scalar.activation(out=std, in_=vs, func=AF.Sqrt, scale=1.0 / D,
                                 bias=eps_sb)
            rstd = stats.tile([128, 1], F32, tag="rstd", name=f"rs{pos}")
            nc.vector.reciprocal(out=rstd, in_=std)
            xg = cpool.tile([128, D], F32, tag="xg", name=f"xg{pos}")
            nc.vector.tensor_mul(xg, xc, gam_sb)
            y = cpool.tile([128, D], F32, tag="x", name=f"y{pos}")
            nc.scalar.activation(out=y, in_=xg, func=AF.Identity, scale=rstd)
            outf = cpool.tile([128, D], F32, tag="xc", name=f"of{pos}")
            nc.vector.tensor_add(outf, y, bet_sb)
            nc.sync.dma_start(out=out_d[pos * 128:(pos + 1) * 128, :], in_=outf)

    nc.compile()
    return nc


_NC_CACHE = {}


def _get_nc():
    if "nc" not in _NC_CACHE:
        _NC_CACHE["nc"] = build_nc_real()
    return _NC_CACHE["nc"]


def _host_inputs(core, q, k, v, Wq, bq, Wk, bk, Wv, bv, Wo, bo, gamma, beta):
    b, j = divmod(core, 4)
    T = tile_set(j)
    qb = np.asarray(q[b], np.float32)
    qrows = qb.reshape(NT, 128, D)[T].reshape(512, D)

    def bf(x):
        return np.ascontiguousarray(np.asarray(x, np.float32)).astype(BF16_NP)

    inp = {
        "qT": bf(qrows.T),
        "kT": bf(np.asarray(k[b], np.float32).T),
        "vT": bf(np.asarray(v[b], np.float32).T),
        "q_res": np.ascontiguousarray(qrows + np.asarray(bo, np.float32)),
        "WqT": bf(np.asarray(Wq).T),
        "WkT": bf(np.asarray(Wk).T),
        "WvT": bf(np.asarray(Wv).T),
        "WoT": bf(np.asarray(Wo).T),
        "bq_p": np.ascontiguousarray(np.asarray(bq, np.float32).reshape(6, 128).T),
        "bk_p": np.ascontiguousarray(np.asarray(bk, np.float32).reshape(6, 128).T),
        "bv_b": np.tile(np.asarray(bv, np.float32), (128, 1)),
        "gamma_b": np.tile(np.asarray(gamma, np.float32), (128, 1)),
        "beta_b": np.tile(np.asarray(beta, np.float32), (128, 1)),
    }
    rows = np.arange(128)[:, None]
    cols = np.arange(128)[None, :]
    # forward mask: per position i, the last 512 columns = key blocks 4i..4i+3
    am4 = np.zeros((128, 4, 512), np.float32)
    for i in range(4):
        t = T[i]
        for joff in range(4):
            jj = 4 * i + joff
            if jj > t:
                am4[:, i, joff * 128:(joff + 1) * 128] = NEG
            elif jj == t:
                am4[:, i, joff * 128:(joff + 1) * 128] = np.where(cols <= rows, 0.0, NEG)
    inp["am4"] = am4
    # transposed mask: per (position i, joff), 0/1 block [128 k, 128 q]
    amT4 = np.ones((128, 4, 4, 128), np.float32)
    for i in range(4):
        t = T[i]
        for joff in range(4):
            jj = 4 * i + joff
            if jj > t:
                amT4[:, i, joff, :] = 0.0
            elif jj == t:
                amT4[:, i, joff, :] = np.where(rows <= cols, 1.0, 0.0)
    inp["amT4"] = np.ascontiguousarray(
        np.broadcast_to(amT4[:, :, :, None, :], (128, 4, 4, 2, 128))
    ).astype(BF16_NP)
    return inp


def kernel(q, k, v, mask, Wq, bq, Wk, bk, Wv, bv, Wo, bo, gamma, beta):
    nc = _get_nc()
    in_maps = [
        _host_inputs(c, q, k, v, Wq, bq, Wk, bk, Wv, bv, Wo, bo, gamma, beta)
        for c in range(N_CORES)
    ]
    res = run_bass_kernel_spmd(nc, in_maps, core_ids=list(range(N_CORES)))

    attn = np.zeros((B, H, S, S), np.float32)
    out = np.empty((B, S, D), np.float32)
    for c in range(N_CORES):
        b, j = divmod(c, 4)
        T = tile_set(j)
        a_o = res.results[c]["attn_o"]
        o_o = res.results[c]["out_o"]
        for i, t in enumerate(T):
            L = (t + 1) * 128
            attn[b, :, t * 128:(t + 1) * 128, 0:L] = \
                a_o[:, i * 128:(i + 1) * 128, 0:L]
            out[b, t * 128:(t + 1) * 128, :] = o_o[i * 128:(i + 1) * 128, :]
    return (out, attn)


# revision 9
# speedup vs baseline: 1.1641x; 1.1641x over previous
"""Multi-head attention + residual + LayerNorm on 8 Trainium2 NeuronCores.

Problem: nn_MultiHeadAttention (B=2, S=2048, D=768, H=12, causal mask),
reference returns (out [2,2048,768], attn [2,12,2048,2048]) in fp32.

Sharding: core = (batch b, 4 q-row tiles of 128). The 16 q-tiles of each batch
are assigned to 4 cores as {j, 7-j, 8+j, 15-j} so causal work/writes balance.
Each core computes all 12 heads for its 512 q rows: Q/K/V projections
(K/V for the full sequence, replicated within a batch group), scores,
softmax, the attn probability tiles (written only for the causal region;
the exact-zero upper triangle is assembled host-side), attn@V, output
projection, residual add and LayerNorm for its rows.

Compute dtype: bf16 operands with fp32 PSUM accumulation; softmax in fp32.

One NEFF runs on all 8 cores (SPMD). Per-core tile sets {t0, 7-t0, 8+t0,
15-t0} (t0 in 0..3) have position-wise worst cases TMAX=[3,7,11,15]; the
program computes worst-case causal widths LMAX per position and host-supplied
additive masks (0 keep / -1e30 drop) encode each core's true tril boundary,
so surplus columns exp() to exact zeros (correct attn output values, zero
contributions to attn@V). Only the last 4 key-blocks of each position can
differ per core, so the masks are small: [128,4,512] forward, [128,4,4,128]
for the transposed path. The reference causal mask is deterministic, so the
`mask` input itself is not re-read on device.
"""

from contextlib import ExitStack

import ml_dtypes
import numpy as np

import concourse.tile as tile
from concourse import bacc, mybir
from concourse.bass_utils import run_bass_kernel_spmd
from concourse.masks import make_identity

B, S, D, H, DK = 2, 2048, 768, 12, 64
NT = S // 128  # 16 q tiles per batch
EPS = 1e-5
N_CORES = 8
NEG = -1.0e30

F32 = mybir.dt.float32
BF16 = mybir.dt.bfloat16
BF16_NP = ml_dtypes.bfloat16
AX = mybir.AxisListType
ALU = mybir.AluOpType
AF = mybir.ActivationFunctionType

TMAX = [3, 7, 11, 15]  # worst-case tile index per position
LMAX = [(t + 1) * 128 for t in TMAX]  # 512, 1024, 1536, 2048


def tile_set(j):
    return sorted([j, 7 - j, 8 + j, 15 - j])


def build_nc_real():
    nc = bacc.Bacc("TRN2", target_bir_lowering=False, debug=False,
                   num_devices=N_CORES)

    qT_d = nc.dram_tensor("qT", [D, 512], BF16, kind="ExternalInput")
    kT_d = nc.dram_tensor("kT", [D, S], BF16, kind="ExternalInput")
    vT_d = nc.dram_tensor("vT", [D, S], BF16, kind="ExternalInput")
    qres_d = nc.dram_tensor("q_res", [512, D], F32, kind="ExternalInput")
    WqT_d = nc.dram_tensor("WqT", [D, D], BF16, kind="ExternalInput")
    WkT_d = nc.dram_tensor("WkT", [D, D], BF16, kind="ExternalInput")
    WvT_d = nc.dram_tensor("WvT", [D, D], BF16, kind="ExternalInput")
    WoT_d = nc.dram_tensor("WoT", [D, D], BF16, kind="ExternalInput")
    bqp_d = nc.dram_tensor("bq_p", [128, 6], F32, kind="ExternalInput")
    bkp_d = nc.dram_tensor("bk_p", [128, 6], F32, kind="ExternalInput")
    bvb_d = nc.dram_tensor("bv_b", [128, D], F32, kind="ExternalInput")
    gam_d = nc.dram_tensor("gamma_b", [128, D], F32, kind="ExternalInput")
    bet_d = nc.dram_tensor("beta_b", [128, D], F32, kind="ExternalInput")
    # forward additive mask: per position, last 512 score columns
    am4_d = nc.dram_tensor("am4", [128, 4, 512], F32, kind="ExternalInput")
    # transposed-path 0/1 multiplicative mask (bf16), per (position, joff),
    # duplicated for the two heads of a pair: [128, pos, joff, 2, 128]
    amT4_d = nc.dram_tensor("amT4", [128, 4, 4, 2, 128], BF16, kind="ExternalInput")

    attn_d = nc.dram_tensor("attn_o", [H, 512, S], F32, kind="ExternalOutput")
    out_d = nc.dram_tensor("out_o", [512, D], F32, kind="ExternalOutput")

    with ExitStack() as top:
        tc = top.enter_context(tile.TileContext(nc))

        consts = top.enter_context(tc.tile_pool(name="consts", bufs=1))
        persist = top.enter_context(tc.tile_pool(name="persist", bufs=1))
        sp = top.enter_context(tc.tile_pool(name="sp", bufs=2, space="PSUM"))
        etp = top.enter_context(tc.tile_pool(name="etp", bufs=1, space="PSUM"))
        opsp = top.enter_context(tc.tile_pool(name="opsp", bufs=2, space="PSUM"))
        stats = top.enter_context(tc.tile_pool(name="stats", bufs=16))

        # ---- consts ----
        WoT_sb = consts.tile([128, 6, D], BF16, tag="wot")
        nc.sync.dma_start(out=WoT_sb, in_=WoT_d.ap().rearrange("(c p) d -> p c d", p=128))
        bq_sb = consts.tile([128, 6], F32, tag="bq")
        nc.sync.dma_start(out=bq_sb, in_=bqp_d.ap())
        bk_sb = consts.tile([128, 6], F32, tag="bk")
        nc.sync.dma_start(out=bk_sb, in_=bkp_d.ap())
        bvb_sb = consts.tile([128, D], F32, tag="bvb")
        nc.sync.dma_start(out=bvb_sb, in_=bvb_d.ap())
        gam_sb = consts.tile([128, D], F32, tag="gam")
        nc.sync.dma_start(out=gam_sb, in_=gam_d.ap())
        bet_sb = consts.tile([128, D], F32, tag="bet")
        nc.sync.dma_start(out=bet_sb, in_=bet_d.ap())
        am4_sb = consts.tile([128, 4, 512], F32, tag="am4")
        nc.sync.dma_start(out=am4_sb, in_=am4_d.ap())
        amT4_sb = consts.tile([128, 4, 4, 2, 128], BF16, tag="amT4")
        nc.sync.dma_start(out=amT4_sb, in_=amT4_d.ap())
        ident_sb = consts.tile([128, 128], BF16, tag="ident")
        make_identity(nc, ident_sb)
        eps_sb = consts.tile([128, 1], F32, tag="eps")
        nc.vector.memset(eps_sb, EPS)
        r_all = consts.tile([128, H, 4], F32, tag="rall")

        # ---- persistent activations ----
        QT_sb = [persist.tile([128, 512], BF16, tag=f"qt{p}", name=f"QT{p}")
                 for p in range(6)]
        KT_sb = [persist.tile([128, S], BF16, tag=f"kt{p}", name=f"KT{p}")
                 for p in range(6)]
        V_sb = [persist.tile([128, D], BF16, tag=f"v{i}", name=f"V{i}")
                for i in range(NT)]
        O_sb = [persist.tile([128, D], BF16, tag=f"o{i}", name=f"O{i}")
                for i in range(4)]

        # ================= phase A: projections =================
        with tc.tile_pool(name="pha", bufs=1) as pha, \
             tc.tile_pool(name="phs", bufs=3) as phs:
            Wq_sb = pha.tile([128, 6, D], BF16, tag="wq")
            nc.sync.dma_start(out=Wq_sb, in_=WqT_d.ap().rearrange("(c p) d -> p c d", p=128))
            Wk_sb = pha.tile([128, 6, D], BF16, tag="wk")
            nc.sync.dma_start(out=Wk_sb, in_=WkT_d.ap().rearrange("(c p) d -> p c d", p=128))
            Wv_sb = pha.tile([128, 6, D], BF16, tag="wv")
            nc.sync.dma_start(out=Wv_sb, in_=WvT_d.ap().rearrange("(c p) d -> p c d", p=128))
            qTi = pha.tile([128, 6, 512], BF16, tag="qTi")
            nc.sync.dma_start(out=qTi, in_=qT_d.ap().rearrange("(c p) n -> p c n", p=128))

            for p in range(6):
                ps = sp.tile([128, 1024], F32, tag="sp", name=f"qps{p}")
                for c in range(6):
                    nc.tensor.matmul(ps[:, 0:512],
                                     lhsT=Wq_sb[:, c, p * 128:(p + 1) * 128],
                                     rhs=qTi[:, c, :],
                                     start=(c == 0), stop=(c == 5))
                nc.scalar.activation(out=QT_sb[p], in_=ps[:, 0:512],
                                     func=AF.Identity, bias=bq_sb[:, p:p + 1])
            for ch in range(4):
                kTi = phs.tile([128, 6, 512], BF16, tag="kTi", name=f"kTi{ch}")
                nc.sync.dma_start(
                    out=kTi,
                    in_=kT_d.ap().rearrange("(c p) n -> p c n", p=128)[:, :, ch * 512:(ch + 1) * 512])
                for p in range(6):
                    ps = sp.tile([128, 1024], F32, tag="sp", name=f"kps{ch}_{p}")
                    for c in range(6):
                        nc.tensor.matmul(ps[:, 0:512],
                                         lhsT=Wk_sb[:, c, p * 128:(p + 1) * 128],
                                         rhs=kTi[:, c, :],
                                         start=(c == 0), stop=(c == 5))
                    nc.scalar.activation(out=KT_sb[p][:, ch * 512:(ch + 1) * 512],
                                         in_=ps[:, 0:512],
                                         func=AF.Identity, bias=bk_sb[:, p:p + 1])
            for ii in range(NT // 2):
                vTi = phs.tile([128, 6, 256], BF16, tag="vTi", name=f"vTi{ii}")
                nc.sync.dma_start(
                    out=vTi,
                    in_=vT_d.ap().rearrange("(c p) n -> p c n", p=128)[:, :, ii * 256:(ii + 1) * 256])
                for s2 in range(2):
                    i = 2 * ii + s2
                    ps = sp.tile([128, 1024], F32, tag="sp", name=f"vps{i}")
                    for c in range(6):
                        nc.tensor.matmul(ps[:, 0:512],
                                         lhsT=vTi[:, c, s2 * 128:(s2 + 1) * 128],
                                         rhs=Wv_sb[:, c, 0:512],
                                         start=(c == 0), stop=(c == 5))
                        nc.tensor.matmul(ps[:, 512:768],
                                         lhsT=vTi[:, c, s2 * 128:(s2 + 1) * 128],
                                         rhs=Wv_sb[:, c, 512:768],
                                         start=(c == 0), stop=(c == 5))
                    nc.vector.tensor_add(V_sb[i], ps[:, 0:768], bvb_sb)

        # ================= phase B: per head-pair, software-pipelined ====
        etpool = top.enter_context(tc.tile_pool(name="etpool", bufs=1))
        efwd = top.enter_context(tc.tile_pool(name="efwd", bufs=4))
        cpool = top.enter_context(tc.tile_pool(name="cpool", bufs=2))

        def emit_fwd(p):
            """forward scores + softmax + attn DMA for both heads of pair p"""
            for hi, h in enumerate((2 * p, 2 * p + 1)):
                for pos in range(4):
                    Lm = LMAX[pos]
                    efw = efwd.tile([128, S], F32, tag="efwd", name=f"ef{h}_{pos}")
                    parts = stats.tile([128, 2], F32, tag="st", name=f"pt{h}_{pos}")
                    nchunk = (Lm + 1023) // 1024
                    for cch in range(nchunk):
                        cl = min(1024, Lm - cch * 1024)
                        ps = sp.tile([128, 1024], F32, tag="sp",
                                     name=f"sps{h}_{pos}_{cch}")
                        for cc in range((cl + 511) // 512):
                            w = min(512, cl - cc * 512)
                            col = cch * 1024 + cc * 512
                            nc.tensor.matmul(
                                ps[:, cc * 512:cc * 512 + w],
                                lhsT=QT_sb[p][hi * 64:(hi + 1) * 64,
                                              pos * 128:(pos + 1) * 128],
                                rhs=KT_sb[p][hi * 64:(hi + 1) * 64, col:col + w],
                                start=True, stop=True)
                        if cch == nchunk - 1:
                            # causal mask on the last 512 columns
                            nc.vector.tensor_add(ps[:, cl - 512:cl],
                                                 ps[:, cl - 512:cl],
                                                 am4_sb[:, pos, :])
                        nc.scalar.activation(out=efw[:, cch * 1024:cch * 1024 + cl],
                                             in_=ps[:, 0:cl], func=AF.Exp,
                                             scale=0.125,
                                             accum_out=parts[:, cch:cch + 1])
                    if nchunk == 2:
                        nc.vector.tensor_add(parts[:, 0:1], parts[:, 0:1],
                                             parts[:, 1:2])
                    rr = r_all[:, h, pos:pos + 1]
                    nc.vector.reciprocal(out=rr, in_=parts[:, 0:1])
                    nc.vector.tensor_scalar_mul(out=efw[:, 0:Lm],
                                                in0=efw[:, 0:Lm], scalar1=rr)
                    nc.sync.dma_start(out=attn_d[h, pos * 128:(pos + 1) * 128, 0:Lm],
                                      in_=efw[:, 0:Lm])

        def emit_et(p):
            """transposed exp tiles (ET) for pair p"""
            ets = []
            for j in range(NT):
                i0 = j // 4  # first position with TMAX >= j
                nv = (4 - i0) * 128
                ps = etp.tile([128, 2, 512], F32, tag="etp", name=f"etp{p}_{j}")
                for hi in range(2):
                    nc.tensor.matmul(ps[:, hi, 0:nv],
                                     lhsT=KT_sb[p][hi * 64:(hi + 1) * 64,
                                                   j * 128:(j + 1) * 128],
                                     rhs=QT_sb[p][hi * 64:(hi + 1) * 64,
                                                  i0 * 128:512],
                                     start=True, stop=True)
                et = etpool.tile([128, 2, nv], BF16, tag=f"etq{i0}",
                                 name=f"et{p}_{j}", bufs=5)
                nc.scalar.activation(out=et, in_=ps[:, :, 0:nv],
                                     func=AF.Exp, scale=0.125)
                # per-core causal 0/1 mask: only block i = j//4 can differ
                nc.vector.tensor_mul(et[:, :, 0:128], et[:, :, 0:128],
                                     amT4_sb[:, i0, j - 4 * i0, :, :])
                ets.append((et, i0))
            return ets

        def emit_attnv(p, ets):
            """attn @ V (forward orientation, unnormalized) for pair p"""
            for hi, h in enumerate((2 * p, 2 * p + 1)):
                ops = opsp.tile([128, 4, 64], F32, tag="opsp", name=f"ops{h}")
                for pos in range(4):
                    tmax = TMAX[pos]
                    for j in range(tmax + 1):
                        et, i0 = ets[j]
                        blk = (pos - i0) * 128
                        nc.tensor.matmul(ops[:, pos, :],
                                         lhsT=et[:, hi, blk:blk + 128],
                                         rhs=V_sb[j][:, h * 64:(h + 1) * 64],
                                         start=(j == 0), stop=(j == tmax))
                for pos in range(4):
                    nc.vector.tensor_scalar_mul(
                        out=O_sb[pos][:, h * 64:(h + 1) * 64],
                        in0=ops[:, pos, :],
                        scalar1=r_all[:, h, pos:pos + 1])

        # pipeline: fwd(p+1) is emitted between ET(p) and attnV(p) so the
        # scheduler can feed PE with scores matmuls while ACT runs ET exps,
        # and feed ACT with fwd exps while PE runs attnV matmuls.
        emit_fwd(0)
        pend = None
        for p in range(6):
            ets = emit_et(p)
            if p + 1 < 6:
                emit_fwd(p + 1)
            emit_attnv(p, ets)

        # ================= phase C: out-proj + residual + LN =================
        for pos in range(4):
            ot_sb = []
            for pch in range(6):
                tp = opsp.tile([128, 128], BF16, tag="opsp", name=f"tp{pos}_{pch}")
                nc.tensor.transpose(out=tp,
                                    in_=O_sb[pos][:, pch * 128:(pch + 1) * 128],
                                    identity=ident_sb)
                ot = cpool.tile([128, 128], BF16, tag=f"ot{pch}",
                                name=f"ot{pos}_{pch}")
                nc.vector.tensor_copy(out=ot, in_=tp)
                ot_sb.append(ot)
            ps = sp.tile([128, 1024], F32, tag="sp", name=f"prj{pos}")
            for pch in range(6):
                nc.tensor.matmul(ps[:, 0:512], lhsT=ot_sb[pch],
                                 rhs=WoT_sb[:, pch, 0:512],
                                 start=(pch == 0), stop=(pch == 5))
                nc.tensor.matmul(ps[:, 512:768], lhsT=ot_sb[pch],
                                 rhs=WoT_sb[:, pch, 512:768],
                                 start=(pch == 0), stop=(pch == 5))
            qr = cpool.tile([128, D], F32, tag="qr", name=f"qr{pos}")
            nc.sync.dma_start(out=qr, in_=qres_d[pos * 128:(pos + 1) * 128, :])
            x = cpool.tile([128, D], F32, tag="x", name=f"x{pos}")
            nc.vector.tensor_add(x, ps[:, 0:768], qr)
            nsum = stats.tile([128, 1], F32, tag="nsum", name=f"ns{pos}")
            nc.vector.tensor_reduce(out=nsum, in_=x, axis=AX.X, op=ALU.add,
                                    negate=True)
            nmean = stats.tile([128, 1], F32, tag="nmean", name=f"nm{pos}")
            nc.vector.tensor_scalar_mul(out=nmean, in0=nsum, scalar1=1.0 / D)
            xc = cpool.tile([128, D], F32, tag="xc", name=f"xc{pos}")
            nc.scalar.activation(out=xc, in_=x, func=AF.Identity, bias=nmean)
            xsq = cpool.tile([128, D], F32, tag="xg", name=f"xq{pos}")
            vs = stats.tile([128, 1], F32, tag="vs", name=f"vs{pos}")
            nc.scalar.activation(out=xsq, in_=xc, func=AF.Square,
                                 accum_out=vs)
            std = stats.tile([128, 1], F32, tag="std", name=f"sd{pos}")
            nc.scalar.activation(out=std, in_=vs, func=AF.Sqrt, scale=1.0 / D,
                                 bias=eps_sb)
            rstd = stats.tile([128, 1], F32, tag="rstd", name=f"rs{pos}")
            nc.vector.reciprocal(out=rstd, in_=std)
            xg = cpool.tile([128, D], F32, tag="xg", name=f"xg{pos}")
            nc.vector.tensor_mul(xg, xc, gam_sb)
            y = cpool.tile([128, D], F32, tag="x", name=f"y{pos}")
            nc.scalar.activation(out=y, in_=xg, func=AF.Identity, scale=rstd)
            outf = cpool.tile([128, D], F32, tag="xc", name=f"of{pos}")
            nc.vector.tensor_add(outf, y, bet_sb)
            nc.sync.dma_start(out=out_d[pos * 128:(pos + 1) * 128, :], in_=outf)

    nc.compile()
    return nc


_NC_CACHE = {}


def _get_nc():
    if "nc" not in _NC_CACHE:
        _NC_CACHE["nc"] = build_nc_real()
    return _NC_CACHE["nc"]


def _host_inputs(core, q, k, v, Wq, bq, Wk, bk, Wv, bv, Wo, bo, gamma, beta):
    b, j = divmod(core, 4)
    T = tile_set(j)
    qb = np.asarray(q[b], np.float32)
    qrows = qb.reshape(NT, 128, D)[T].reshape(512, D)

    def bf(x):
        return np.ascontiguousarray(np.asarray(x, np.float32)).astype(BF16_NP)

    inp = {
        "qT": bf(qrows.T),
        "kT": bf(np.asarray(k[b], np.float32).T),
        "vT": bf(np.asarray(v[b], np.float32).T),
        "q_res": np.ascontiguousarray(qrows + np.asarray(bo, np.float32)),
        "WqT": bf(np.asarray(Wq).T),
        "WkT": bf(np.asarray(Wk).T),
        "WvT": bf(np.asarray(Wv).T),
        "WoT": bf(np.asarray(Wo).T),
        "bq_p": np.ascontiguousarray(np.asarray(bq, np.float32).reshape(6, 128).T),
        "bk_p": np.ascontiguousarray(np.asarray(bk, np.float32).reshape(6, 128).T),
        "bv_b": np.tile(np.asarray(bv, np.float32), (128, 1)),
        "gamma_b": np.tile(np.asarray(gamma, np.float32), (128, 1)),
        "beta_b": np.tile(np.asarray(beta, np.float32), (128, 1)),
    }
    rows = np.arange(128)[:, None]
    cols = np.arange(128)[None, :]
    # forward mask: per position i, the last 512 columns = key blocks 4i..4i+3
    am4 = np.zeros((128, 4, 512), np.float32)
    for i in range(4):
        t = T[i]
        for joff in range(4):
            jj = 4 * i + joff
            if jj > t:
                am4[:, i, joff * 128:(joff + 1) * 128] = NEG
            elif jj == t:
                am4[:, i, joff * 128:(joff + 1) * 128] = np.where(cols <= rows, 0.0, NEG)
    inp["am4"] = am4
    # transposed mask: per (position i, joff), 0/1 block [128 k, 128 q]
    amT4 = np.ones((128, 4, 4, 128), np.float32)
    for i in range(4):
        t = T[i]
        for joff in range(4):
            jj = 4 * i + joff
            if jj > t:
                amT4[:, i, joff, :] = 0.0
            elif jj == t:
                amT4[:, i, joff, :] = np.where(rows <= cols, 1.0, 0.0)
    inp["amT4"] = np.ascontiguousarray(
        np.broadcast_to(amT4[:, :, :, None, :], (128, 4, 4, 2, 128))
    ).astype(BF16_NP)
    return inp


def kernel(q, k, v, mask, Wq, bq, Wk, bk, Wv, bv, Wo, bo, gamma, beta):
    nc = _get_nc()
    in_maps = [
        _host_inputs(c, q, k, v, Wq, bq, Wk, bk, Wv, bv, Wo, bo, gamma, beta)
        for c in range(N_CORES)
    ]
    res = run_bass_kernel_spmd(nc, in_maps, core_ids=list(range(N_CORES)))

    attn = np.zeros((B, H, S, S), np.float32)
    out = np.empty((B, S, D), np.float32)
    for c in range(N_CORES):
        b, j = divmod(c, 4)
        T = tile_set(j)
        a_o = res.results[c]["attn_o"]
        o_o = res.results[c]["out_o"]
        for i, t in enumerate(T):
            L = (t + 1) * 128
            attn[b, :, t * 128:(t + 1) * 128, 0:L] = \
                a_o[:, i * 128:(i + 1) * 128, 0:L]
            out[b, t * 128:(t + 1) * 128, :] = o_o[i * 128:(i + 1) * 128, :]
    return (out, attn)
